# revision 34
# baseline (speedup 1.0000x reference)
"""DeltaNet (gated linear attention) Trainium2 kernel, 8-core SPMD.

Sharding: core c handles batch b=c//2 and head-half hg=c%2 (8 of 16 heads).
The sequential scan is reformulated as chunked linear attention (chunk 256):
  S_t = lam*S_{t-1} + g_t*(k_t v_t^T);  y_t = (q_t^T S_t)/(q_t.s_t + eps)
Per chunk, in lam^{-j}-primed coordinates:
  Yt[e,i] = q_i^T T_aug + sum_j ATm[j,i] * Vc_aug[j,e]
  y_i = Yt[0:64,i] / (Yt[64,i] + EPS*lam^-i)
  T_next = lam^C * (T + pk_chunk^T @ Vc_aug)
where ATm[j,i] = (pk_j . pq_i) masked causal, Vc_aug[j,:] = [v_j*clip*c_j | c_j],
c_j = g_j * lam^-j * mask_j. Output proj partials pair-ReduceScatter'ed, then
residual + LayerNorm on each core's half of the rows.
"""

import os
import sys

for _p in (
    "/root/.axon_site",
    "/root/.axon_site/_ro/trn_rl_repo",
    "/root/.axon_site/_ro/pypackages",
    "/opt/trn_rl_repo",
):
    if os.path.isdir(_p) and _p not in sys.path:
        sys.path.append(_p)

import numpy as np

B, H, NH, D = 4, 1024, 16, 64
EPS, V_CLIP, LN_EPS = 1e-6, 2.0, 1e-5
NHL = 8          # local heads per core
EW = NHL * D     # 512 local e-width
CO = 256         # outer chunk length
S_FULL = 2048


def build(S=S_FULL):
    from concourse import bacc, mybir
    from concourse.tile import TileContext

    dt = mybir.dt
    f32, f32r, bf16 = dt.float32, dt.float32r, dt.bfloat16

    NTC = S // 512   # 512-col projection chunks
    NTT = S // 128   # 128-row tiles
    NCH = S // CO    # outer chunks
    HS = S // 2      # rows per core after reduce-scatter

    nc = bacc.Bacc("TRN2", target_bir_lowering=False)

    xT_e = nc.declare_dram_parameter("xT", [H, S], f32r, isOutput=False)
    xg_e = nc.declare_dram_parameter("xg", [HS, H], f32, isOutput=False)
    wqkv_e = nc.declare_dram_parameter("wqkv", [H, 3 * EW], f32r, isOutput=False)
    woT_e = nc.declare_dram_parameter("woT", [EW, H], f32r, isOutput=False)
    gw_e = nc.declare_dram_parameter("gw", [128, 4, 16], bf16, isOutput=False)
    gb_e = nc.declare_dram_parameter("gb", [8, 1], f32, isOutput=False)
    maskT_e = nc.declare_dram_parameter("maskT", [128, NTT], f32, isOutput=False)
    lamInv_e = nc.declare_dram_parameter("lamInv", [128, 2, 8], f32, isOutput=False)
    lamC_e = nc.declare_dram_parameter("lamC", [128, 4, 65], f32, isOutput=False)
    eps32_e = nc.declare_dram_parameter("eps32", [128, 2, CO], f32, isOutput=False)
    maskF_e = nc.declare_dram_parameter("maskF", [128, 384], f32, isOutput=False)
    ident8_e = nc.declare_dram_parameter("ident8", [8, 8], f32, isOutput=False)
    ones64_e = nc.declare_dram_parameter("ones64", [128, 64], f32r, isOutput=False)
    gamma_e = nc.declare_dram_parameter("gamma", [128, H], f32, isOutput=False)
    beta_e = nc.declare_dram_parameter("beta", [128, H], f32, isOutput=False)
    out_e = nc.declare_dram_parameter("out", [HS, H], f32, isOutput=True)

    with TileContext(nc) as tc:
        with (
            tc.tile_pool(name="persist", bufs=1) as pp,
            tc.tile_pool(name="dram", bufs=1, space="DRAM") as dram,
        ):
            # ---- persistent SBUF tensors ----
            pqT = pp.tile([128, 4, S], bf16)      # [e%128, eb, t]
            pkT = pp.tile([128, 4, S], bf16)
            knat = pp.tile([128, NTT, EW], bf16)  # [t%128, tt, e]
            vnat = pp.tile([128, NTT, EW], bf16)
            clip_all = pp.tile([128, NTT, 8], f32)
            g8 = pp.tile([8, S], f32)
            T_all = pp.tile([128, 4, 65], f32)    # state: head h -> rows (h%2)*64, idx h//2
            wo_sb = pp.tile([128, 4, H], f32r)
            gamma_sb = pp.tile([128, H], f32)
            beta_sb = pp.tile([128, H], f32)
            gw_sb = pp.tile([128, 4, 16], bf16)
            gb_sb = pp.tile([8, 1], f32)
            maskT_sb = pp.tile([128, NTT], f32)
            lamInv_sb = pp.tile([128, 2, 8], f32)
            lamC_sb = pp.tile([128, 4, 65], f32)
            eps32_sb = pp.tile([128, 2, CO], f32)
            maskF_sb = pp.tile([128, 384], f32)
            ident8_sb = pp.tile([8, 8], f32)
            ones64_sb = pp.tile([128, 64], f32r)

            nc.sync.dma_start(wo_sb[:], woT_e.rearrange("(eb p) o -> p eb o", p=128))
            nc.sync.dma_start(gamma_sb[:], gamma_e[:])
            nc.sync.dma_start(beta_sb[:], beta_e[:])
            nc.sync.dma_start(gw_sb[:], gw_e[:])
            nc.sync.dma_start(gb_sb[:], gb_e[:])
            nc.sync.dma_start(maskT_sb[:], maskT_e[:])
            nc.sync.dma_start(lamInv_sb[:], lamInv_e[:])
            nc.sync.dma_start(lamC_sb[:], lamC_e[:])
            nc.sync.dma_start(eps32_sb[:], eps32_e[:])
            nc.sync.dma_start(maskF_sb[:], maskF_e[:])
            nc.sync.dma_start(ident8_sb[:], ident8_e[:])
            nc.sync.dma_start(ones64_sb[:], ones64_e[:])
            nc.gpsimd.memset(T_all[:], 0.0)
            # selector weights for the denominator matmuls: czel{0,1}[:, h, :]
            # has c_half{0,1} for head h in column h, zeros elsewhere; sv[:, eb, :]
            # carries the running svec state for the two heads of e-block eb.
            czel = pp.tile([128, 2, 8, 128], bf16)
            sv = pp.tile([128, 4, 128], bf16)
            nc.gpsimd.memset(czel[:], 0.0)
            nc.gpsimd.memset(sv[:], 0.0)

            ob_dram = dram.tile([S, H], f32)
            rs_dram = dram.tile([HS, H], f32)

            # ================= phase A: projections =================
            with (
                tc.tile_pool(name="pa_w", bufs=1) as paw,
                tc.tile_pool(name="pa_sbuf", bufs=2) as pa,
                tc.tile_pool(name="pa_tmp", bufs=3) as pat,
                tc.tile_pool(name="pa_psum", bufs=3, space="PSUM") as pap,
                tc.tile_pool(name="pa_psum2", bufs=2, space="PSUM") as pap2,
                tc.tile_pool(name="pa_psum3", bufs=1, space="PSUM") as pap3,
            ):
                w_sb = paw.tile([128, 8, 3 * EW], f32r, tag="w_sb")
                nc.sync.dma_start(
                    w_sb[:], wqkv_e.rearrange("(hb p) e -> p hb e", p=128)
                )

                def phi(dst, src_ps, tmp_pool):
                    ex = tmp_pool.tile([128, 512], f32, tag="phi_ex")
                    rl = tmp_pool.tile([128, 512], f32, tag="phi_rl")
                    nc.scalar.activation(ex[:], src_ps, mybir.ActivationFunctionType.Exp)
                    nc.scalar.activation(rl[:], src_ps, mybir.ActivationFunctionType.Relu)
                    # dst = min(exp, 1) + relu  (== elu(z)+1)
                    nc.vector.scalar_tensor_tensor(
                        dst, ex[:], 1.0, rl[:],
                        mybir.AluOpType.min, mybir.AluOpType.add,
                    )

                for tct in range(NTC):
                    ts = tct * 512
                    xt = pa.tile([128, 8, 512], f32r, tag="xT")
                    nc.sync.dma_start(
                        xt[:],
                        xT_e.rearrange("(hb p) t -> p hb t", p=128)[:, :, ts : ts + 512],
                    )
                    # qT / kT: [e(128=2 heads), t(512)]
                    for eb in range(4):
                        for qk in range(2):
                            ps = pap.tile([128, 512], f32, tag="qk_ps")
                            for hb in range(8):
                                nc.tensor.matmul(
                                    ps[:],
                                    w_sb[:, hb, qk * EW + eb * 128 : qk * EW + eb * 128 + 128],
                                    xt[:, hb, :],
                                    start=(hb == 0),
                                    stop=(hb == 7),
                                )
                            dst = (pqT if qk == 0 else pkT)[:, eb, ts : ts + 512]
                            phi(dst, ps[:], pat)
                    # k natural / v natural: [t(128), e(512)]
                    for sub in range(4):
                        tt = tct * 4 + sub
                        kn = pap2.tile([128, 512], f32, tag="kn_ps")
                        vn = pap2.tile([128, 512], f32, tag="vn_ps")
                        for hb in range(8):
                            lhs = xt[:, hb, sub * 128 : sub * 128 + 128]
                            nc.tensor.matmul(
                                kn[:], lhs, w_sb[:, hb, EW : 2 * EW],
                                start=(hb == 0), stop=(hb == 7),
                            )
                            nc.tensor.matmul(
                                vn[:], lhs, w_sb[:, hb, 2 * EW : 3 * EW],
                                start=(hb == 0), stop=(hb == 7),
                            )
                        phi(knat[:, tt, :], kn[:], pat)
                        nc.vector.tensor_copy(vnat[:, tt, :], vn[:])
                        # v row norms per head -> clip scale
                        sq = pat.tile([128, 512], f32, tag="sq")
                        nc.scalar.activation(sq[:], vn[:], mybir.ActivationFunctionType.Square)
                        vn2 = pat.tile([128, 8], f32, tag="vn2")
                        nc.vector.reduce_sum(
                            vn2[:], sq[:].rearrange("p (h d) -> p h d", d=64),
                            axis=mybir.AxisListType.X,
                        )
                        vnorm = pat.tile([128, 8], f32, tag="vnorm")
                        nc.scalar.activation(vnorm[:], vn2[:], mybir.ActivationFunctionType.Sqrt)
                        vne = pat.tile([128, 8], f32, tag="vne")
                        nc.vector.tensor_scalar_add(vne[:], vnorm[:], EPS)
                        vri = pat.tile([128, 8], f32, tag="vri")
                        nc.vector.reciprocal(vri[:], vne[:])
                        nc.vector.tensor_scalar(
                            clip_all[:, tt, :], vri[:], V_CLIP, 1.0,
                            mybir.AluOpType.mult, mybir.AluOpType.min,
                        )
                    # gates: [8, 512] logits accumulated over e-blocks, q+k parts
                    gp = pap3.tile([8, 512], f32, tag="g_ps")
                    for eb in range(4):
                        nc.tensor.matmul(
                            gp[:], gw_sb[:, eb, 0:8], pqT[:, eb, ts : ts + 512],
                            start=(eb == 0), stop=False,
                        )
                        nc.tensor.matmul(
                            gp[:], gw_sb[:, eb, 8:16], pkT[:, eb, ts : ts + 512],
                            start=False, stop=(eb == 3),
                        )
                    nc.scalar.activation(
                        g8[:, ts : ts + 512], gp[:],
                        mybir.ActivationFunctionType.Sigmoid, bias=gb_sb[:],
                    )

            # ================= phase B: recurrence + phase C: out proj =================
            with (
                tc.tile_pool(name="pb_sbuf", bufs=2) as pb,
                tc.tile_pool(name="pb_small", bufs=3) as pbs,
                tc.tile_pool(name="pb_big", bufs=1) as pbb,
                tc.tile_pool(name="pb_atm", bufs=10) as patm,
                tc.tile_pool(name="pb_vc", bufs=4) as pvc,
                tc.tile_pool(name="at_psum", bufs=2, space="PSUM") as p_at,
                tc.tile_pool(name="yt_psum", bufs=2, space="PSUM") as p_yt,
                tc.tile_pool(name="dn_psum", bufs=1, space="PSUM") as p_dn,
                tc.tile_pool(name="sd_psum", bufs=1, space="PSUM") as p_sd,
                tc.tile_pool(name="o_psum", bufs=1, space="PSUM") as p_o,
                tc.tile_pool(name="rb_psum", bufs=1, space="PSUM") as p_rb,
            ):
                yT = pbb.tile([128, 4, S], f32r)
                for ch in range(NCH):
                    cs0 = ch * CO
                    # bf16 copy of state for matmul lhsT
                    T16 = pbs.tile([128, 4, 65], bf16, tag="T16")
                    nc.vector.tensor_copy(T16[:], T_all[:])

                    # per-half gate columns c and c*clip, and Vc_aug
                    Vc = []
                    c_halves = []
                    for hf in range(2):
                        tt = 2 * ch + hf
                        gt_ps = p_dn.tile([128, 512], f32, tag="dn_ps", name="gt_ps")
                        nc.tensor.transpose(
                            gt_ps[:, 0:8], g8[:, tt * 128 : tt * 128 + 128], ident8_sb[:]
                        )
                        c_pre = pbs.tile([128, 8], f32, tag="c_pre")
                        nc.vector.tensor_tensor(
                            c_pre[:], gt_ps[:, 0:8], lamInv_sb[:, hf, :], mybir.AluOpType.mult
                        )
                        c_all = pbs.tile([128, 8], f32, tag="c_all")
                        nc.vector.tensor_scalar_mul(
                            c_all[:], c_pre[:], maskT_sb[:, tt : tt + 1]
                        )
                        cs_all = pbs.tile([128, 8], f32, tag="cs_all")
                        nc.vector.tensor_tensor(
                            cs_all[:], c_all[:], clip_all[:, tt, :], mybir.AluOpType.mult
                        )
                        vc = pvc.tile([128, 8, 65], bf16, tag="vc")
                        for h in range(NHL):
                            nc.vector.tensor_scalar_mul(
                                vc[:, h, 0:64], vnat[:, tt, h * 64 : h * 64 + 64],
                                cs_all[:, h : h + 1],
                            )
                        nc.vector.tensor_copy(vc[:, :, 64:65], c_all[:].unsqueeze(2))
                        Vc.append(vc)
                        c_halves.append(c_all)
                        for h in range(NHL):
                            hx = h % 4
                            nc.vector.tensor_copy(
                                czel[:, hf, h, 32 * hx : 32 * hx + 1],
                                c_all[:, h : h + 1],
                            )
                    for eb in range(4):
                        ha, hb = 2 * eb, 2 * eb + 1
                        nc.vector.tensor_copy(
                            sv[0:64, eb, 32 * (ha % 4) : 32 * (ha % 4) + 1],
                            T16[0:64, eb, 64:65],
                        )
                        nc.vector.tensor_copy(
                            sv[64:128, eb, 32 * (hb % 4) : 32 * (hb % 4) + 1],
                            T16[64:128, eb, 64:65],
                        )

                    # A^T matmuls + causal mask, per head
                    atm = []
                    for h in range(NHL):
                        e0, eb = (h % 2) * 64, h // 2
                        at_ps = p_at.tile([128, 384], f32, tag="at_ps")
                        nc.tensor.matmul(
                            at_ps[:, 0:256],
                            pkT[e0 : e0 + 64, eb, cs0 : cs0 + 128],
                            pqT[e0 : e0 + 64, eb, cs0 : cs0 + 256],
                            start=True, stop=True,
                        )
                        nc.tensor.matmul(
                            at_ps[:, 256:384],
                            pkT[e0 : e0 + 64, eb, cs0 + 128 : cs0 + 256],
                            pqT[e0 : e0 + 64, eb, cs0 + 128 : cs0 + 256],
                            start=True, stop=True,
                        )
                        am = patm.tile([128, 384], bf16, tag="atm")
                        nc.vector.tensor_tensor(
                            am[:], at_ps[:], maskF_sb[:], mybir.AluOpType.mult
                        )
                        atm.append(am)

                    # denominators: head h -> group g=h//4, psum row 32*(h%4):
                    #   dn[32hx, i] = sum_j c_j*ATm_h[j,i] + svec_h . q_i
                    dn_ps = p_dn.tile([128, 512], f32, tag="dn_ps")
                    for h in range(NHL):
                        go = (h // 4) * 256
                        nc.tensor.matmul(
                            dn_ps[:, go : go + 256], czel[:, 0, h, :], atm[h][:, 0:256],
                            start=(h == 0), stop=False,
                        )
                        nc.tensor.matmul(
                            dn_ps[:, go + 128 : go + 256], czel[:, 1, h, :],
                            atm[h][:, 256:384],
                            start=False, stop=False,
                        )
                    for eb in range(4):
                        go = ((2 * eb) // 4) * 256
                        nc.tensor.matmul(
                            dn_ps[:, go : go + 256], sv[:, eb, :],
                            pqT[:, eb, cs0 : cs0 + 256],
                            start=False, stop=(eb == 3),
                        )
                    rd = []
                    for g in range(2):
                        dne = pbs.tile([128, CO], f32, tag="dne")
                        nc.vector.tensor_tensor(
                            dne[:], dn_ps[:, g * 256 : g * 256 + 256],
                            eps32_sb[:, g, :],
                            mybir.AluOpType.add,
                        )
                        r = pbs.tile([128, CO], f32r, tag="rd")
                        with nc.allow_low_precision(reason="fp32r is full precision here"):
                            nc.vector.reciprocal(r[:], dne[:])
                        rhi = pbs.tile([64, CO], f32r, tag="rdhi")
                        with nc.allow_low_precision(reason="fp32r is full precision here"):
                            nc.vector.tensor_copy(rhi[:], r[64:128, :])
                        rd.append((r, rhi))

                    # numerators per head, then divide
                    for h in range(NHL):
                        e0, eb = (h % 2) * 64, h // 2
                        yt_ps = p_yt.tile([64, 512], f32, tag="yt_ps")
                        nc.tensor.matmul(
                            yt_ps[:, 0:256],
                            T16[e0 : e0 + 64, eb, 0:64],
                            pqT[e0 : e0 + 64, eb, cs0 : cs0 + 256],
                            start=True, stop=False,
                        )
                        nc.tensor.matmul(
                            yt_ps[:, 0:256], Vc[0][:, h, 0:64], atm[h][:, 0:256],
                            start=False, stop=False,
                        )
                        nc.tensor.matmul(
                            yt_ps[:, 128:256], Vc[1][:, h, 0:64], atm[h][:, 256:384],
                            start=False, stop=True,
                        )
                        nm = pbs.tile([64, CO], f32, tag="nm")
                        nc.scalar.copy(nm[:], yt_ps[:, 0:256])
                        rlo, rhi = rd[h // 4]
                        rsrc = rlo if h % 4 < 2 else rhi
                        hx = 32 * (h % 4) if h % 4 < 2 else 32 * (h % 4 - 2)
                        rdb_ps = p_rb.tile([64, 512], f32, tag="rdb_ps")
                        nc.tensor.matmul(
                            rdb_ps[:, 0:256], ones64_sb[hx : hx + 1, :],
                            rsrc[hx : hx + 1, :],
                            start=True, stop=True,
                        )
                        nc.vector.tensor_tensor(
                            yT[e0 : e0 + 64, eb, cs0 : cs0 + 256],
                            nm[:], rdb_ps[:, 0:256], mybir.AluOpType.mult,
                        )

                    # state update: T = lam^CO * (T + knat^T @ Vc_aug)
                    sd_ps = p_sd.tile([128, 4, 128], f32, tag="sd_ps")
                    for h in range(NHL):
                        e0, eb = (h % 2) * 64, h // 2
                        for hf in range(2):
                            tt = 2 * ch + hf
                            nc.tensor.matmul(
                                sd_ps[e0 : e0 + 64, eb, 0:65],
                                knat[:, tt, h * 64 : h * 64 + 64],
                                Vc[hf][:, h, :],
                                start=(hf == 0), stop=(hf == 1),
                            )
                    t_tmp = pbs.tile([128, 4, 65], f32, tag="t_tmp")
                    nc.vector.tensor_tensor(
                        t_tmp[:], sd_ps[:, :, 0:65], T_all[:], mybir.AluOpType.add
                    )
                    nc.vector.tensor_tensor(
                        T_all[:], t_tmp[:], lamC_sb[:], mybir.AluOpType.mult
                    )

                    # ---- phase C: output projection for this chunk's 2 t-tiles ----
                    for hf in range(2):
                        tt = 2 * ch + hf
                        t0 = tt * 128
                        for oh in range(2):
                            o_ps = p_o.tile([128, 512], f32, tag="o_ps")
                            for eb in range(4):
                                nc.tensor.matmul(
                                    o_ps[:],
                                    yT[:, eb, t0 : t0 + 128],
                                    wo_sb[:, eb, oh * 512 : oh * 512 + 512],
                                    start=(eb == 0), stop=(eb == 3),
                                )
                            ocp = pb.tile([128, 512], f32, tag="ocp")
                            nc.scalar.copy(ocp[:], o_ps[:])
                            nc.sync.dma_start(
                                ob_dram[t0 : t0 + 128, oh * 512 : oh * 512 + 512], ocp[:]
                            )

            # ================= reduce-scatter + residual + LayerNorm =================
            nc.gpsimd.collective_compute(
                "ReduceScatter",
                mybir.AluOpType.add,
                replica_groups=[[0, 1], [2, 3], [4, 5], [6, 7]],
                ins=[ob_dram.opt()],
                outs=[rs_dram.opt()],
            )
            with (
                tc.tile_pool(name="ln_sbuf", bufs=2) as pl,
                tc.tile_pool(name="ln_small", bufs=3) as pls,
            ):
                for rt in range(HS // 128):
                    zin = pl.tile([128, H], f32, tag="zin")
                    nc.sync.dma_start(zin[:], rs_dram[rt * 128 : rt * 128 + 128, :])
                    xgt = pl.tile([128, H], f32, tag="xgt")
                    nc.sync.dma_start(xgt[:], xg_e[rt * 128 : rt * 128 + 128, :])
                    z = pl.tile([128, H], f32, tag="z")
                    nc.vector.tensor_tensor(z[:], zin[:], xgt[:], mybir.AluOpType.add)
                    scr = pl.tile([128, H], f32, tag="scr")
                    ssum = pls.tile([128, 1], f32, tag="ssum")
                    ssq = pls.tile([128, 1], f32, tag="ssq")
                    nc.scalar.activation(
                        scr[:], z[:], mybir.ActivationFunctionType.Identity,
                        accum_out=ssum[:],
                    )
                    nc.scalar.activation(
                        scr[:], z[:], mybir.ActivationFunctionType.Square,
                        accum_out=ssq[:],
                    )
                    mu = pls.tile([128, 1], f32, tag="mu")
                    nc.vector.tensor_scalar_mul(mu[:], ssum[:], 1.0 / H)
                    msq = pls.tile([128, 1], f32, tag="msq")
                    nc.vector.tensor_scalar_mul(msq[:], ssq[:], 1.0 / H)
                    mu2 = pls.tile([128, 1], f32, tag="mu2")
                    nc.scalar.activation(mu2[:], mu[:], mybir.ActivationFunctionType.Square)
                    var = pls.tile([128, 1], f32, tag="var")
                    nc.vector.tensor_tensor(var[:], msq[:], mu2[:], mybir.AluOpType.subtract)
                    vare = pls.tile([128, 1], f32, tag="vare")
                    nc.vector.tensor_scalar_add(vare[:], var[:], LN_EPS)
                    sd = pls.tile([128, 1], f32, tag="sd")
                    nc.scalar.activation(sd[:], vare[:], mybir.ActivationFunctionType.Sqrt)
                    rstd = pls.tile([128, 1], f32, tag="rstd")
                    nc.vector.reciprocal(rstd[:], sd[:])
                    zn = pl.tile([128, H], f32, tag="zn")
                    nc.vector.tensor_scalar(
                        zn[:], z[:], mu[:], rstd[:],
                        mybir.AluOpType.subtract, mybir.AluOpType.mult,
                    )
                    zg = pl.tile([128, H], f32, tag="zg")
                    nc.gpsimd.tensor_tensor(zg[:], zn[:], gamma_sb[:], mybir.AluOpType.mult)
                    of = pl.tile([128, H], f32, tag="of")
                    nc.gpsimd.tensor_tensor(of[:], zg[:], beta_sb[:], mybir.AluOpType.add)
                    nc.sync.dma_start(out_e[rt * 128 : rt * 128 + 128, :], of[:])

    nc.finalize()
    return nc


def _softplus(x):
    return np.log1p(np.exp(-np.abs(x))) + np.maximum(x, 0.0)


def host_prep(inputs, S=S_FULL):
    """Build the 8 per-core input maps from full inputs."""
    import ml_dtypes

    x = np.asarray(inputs["x"], np.float32)
    mask = np.asarray(inputs["mask"])
    Wq = np.asarray(inputs["Wq"], np.float32)
    Wk = np.asarray(inputs["Wk"], np.float32)
    Wv = np.asarray(inputs["Wv"], np.float32)
    Wo = np.asarray(inputs["Wo"], np.float32)
    bo = np.asarray(inputs["bo"], np.float32)
    theta = np.asarray(inputs["decay_theta"], np.float32)
    gate_w = np.asarray(inputs["gate_w"], np.float32)
    gate_b = np.asarray(inputs["gate_b"], np.float32)
    ln_g = np.asarray(inputs["ln_gamma"], np.float32)
    ln_b = np.asarray(inputs["ln_beta"], np.float32)

    NTT = S // 128
    lam = np.exp(-_softplus(theta.astype(np.float64)))  # [NH]

    maskF = np.zeros((128, 384), np.float32)
    jj = np.arange(128)
    maskF[:, 0:256] = (np.arange(256)[None, :] >= jj[:, None]).astype(np.float32)
    maskF[:, 256:384] = (np.arange(128)[None, :] >= jj[:, None]).astype(np.float32)
    ident8 = np.eye(8, dtype=np.float32)
    gamma_full = np.broadcast_to(ln_g, (128, H)).copy()
    beta_full = np.broadcast_to(ln_b, (128, H)).copy()

    per_hg = []
    for hg in range(2):
        sl = slice(hg * EW, (hg + 1) * EW)
        heads = np.arange(hg * NHL, (hg + 1) * NHL)
        lamh = lam[heads]  # [8]
        wqkv = np.concatenate([Wq[sl].T, Wk[sl].T, Wv[sl].T], axis=1)
        wqkv = np.ascontiguousarray(wqkv, np.float32)
        woT = np.ascontiguousarray(Wo[:, sl].T, np.float32)
        gw = np.zeros((128, 4, 16), np.float32)
        for eb in range(4):
            for p in range(128):
                lh = 2 * eb + p // 64
                d = p % 64
                gh = hg * NHL + lh
                gw[p, eb, lh] = gate_w[gh, d]
                gw[p, eb, 8 + lh] = gate_w[gh, D + d]
        gb = gate_b[heads].reshape(8, 1).astype(np.float32)
        j = np.arange(128)
        lamInv = np.zeros((128, 2, 8), np.float32)
        for hf in range(2):
            lamInv[:, hf, :] = lamh[None, :] ** (-(j[:, None] + 128.0 * hf))
        lamC = np.zeros((128, 4, 65), np.float32)
        for h in range(NHL):
            e0, eb = (h % 2) * 64, h // 2
            lamC[e0 : e0 + 64, eb, :] = lamh[h] ** CO
        i = np.arange(CO)
        # unused rows get 1.0 so the batched reciprocal stays finite there
        eps32 = np.ones((128, 2, CO), np.float32)
        for g in range(2):
            for hx in range(4):
                eps32[32 * hx, g, :] = EPS * lamh[g * 4 + hx] ** (-i)
        per_hg.append(
            dict(
                wqkv=wqkv,
                woT=woT,
                gw=gw.astype(ml_dtypes.bfloat16),
                gb=gb,
                lamInv=lamInv,
                lamC=lamC,
                eps32=eps32,
            )
        )

    in_maps = []
    for c in range(8):
        b, hg = c // 2, c % 2
        xb = x[b, :S]
        m = dict(per_hg[hg])
        m["xT"] = np.ascontiguousarray(xb.T)
        m["xg"] = np.ascontiguousarray(xb[hg * (S // 2) : (hg + 1) * (S // 2)] + bo[None, :])
        m["maskT"] = np.ascontiguousarray(
            mask[b, :S].astype(np.float32).reshape(NTT, 128).T
        )
        m["maskF"] = maskF
        m["ident8"] = ident8
        m["ones64"] = np.ones((128, 64), np.float32)
        m["gamma"] = gamma_full
        m["beta"] = beta_full
        in_maps.append(m)
    return in_maps


_CACHE = {}


def kernel(**inputs):
    from concourse import bass2jax

    S = inputs["x"].shape[1]
    if S not in _CACHE:
        _CACHE[S] = build(S)
    nc = _CACHE[S]
    in_maps = host_prep(inputs, S)
    results = bass2jax.run_bass_via_pjrt(nc, in_maps, n_cores=8)
    out = np.empty((B, S, H), np.float32)
    for b in range(B):
        out[b, : S // 2] = results[2 * b]["out"]
        out[b, S // 2 :] = results[2 * b + 1]["out"]
    return out
